# revision 26
# baseline (speedup 1.0000x reference)
"""Grouped-expert FFN (MoE) Trainium2 kernel.

Problem: E=64 experts, each x[1024,512] @ w1[512,2048] -> +b1 -> gelu(erf)
-> @ w2[2048,512] -> +b2, rows >= valid_load[e] zeroed.

Strategy (v3 — bf16, piece-level load balancing, LP-optimized packing):
 - All matmul operands in bf16 (PE streams 1 col/cycle, same peak as fp32r,
   and LDWEIGHTS hides under the 64-deep PE reorder window; DMA halves).
   PSUM accumulation and biases stay fp32; rel err ~5e-3 << 2e-2 gate.
 - Work is split into per-expert column *pieces* of <= 512 columns (one
   PSUM bank). Each (core, slot) runs one piece; the host duplicates the
   expert's weights into that slot's DRAM input, so pieces of one expert
   can land on any cores. The SPMD cost is sum over slot ranks of the
   rank width (every core runs every rank at its compile-time width), so
   the packing problem is: assign experts to bins (8 bins per rank),
   minimize sum of rank widths subject to each expert's bins covering
   its valid columns. Solved with an LP over rank widths inside a
   simulated-annealing search over the bin topology (deterministic
   seed, time-bounded, falls back to sorted-deal if scipy is missing).
 - Host transposes x per piece (xT [D,W]) so the device contracts over D
   with zero on-chip transposes; tokens ride the moving/free axis through
   both GEMMs; biases land on the partition axis -> free via ACT bias.
 - w1/w2 are loaded in halves on the two HWDGE rings (sync+ACT) so the
   first GEMM of a slot can start at half-load and ring load stays even.
 - Host assembles the full fp32 output (invalid rows zeroed).
"""

import time

import numpy as np
import ml_dtypes

import concourse.bass as bass
import concourse.bacc as bacc
import concourse.tile as tile
from concourse import mybir
from concourse.bass_utils import run_bass_kernel_spmd

E, CAP, D, H = 64, 1024, 512, 2048
N_CORES = 8
GMAX = 512                       # max piece width (= one PSUM bank of fp32)
KT1 = D // 128                   # 4   contraction tiles of GEMM1
MT1 = H // 128                   # 16  output partition tiles of GEMM1
KT2 = H // 128                   # 16  contraction tiles of GEMM2
MT2 = D // 128                   # 4   output partition tiles of GEMM2

F32 = mybir.dt.float32
BF16 = mybir.dt.bfloat16
NPBF16 = ml_dtypes.bfloat16

_PROGRAM_CACHE: dict[tuple, object] = {}
LAST_RESULT = None               # test harness introspection


def _deal_jobs(va):
    """Equal-split pieces of <=GMAX, dealt sorted-desc 8 per rank.
    Returns topology N[expert_idx, rank] = bin count."""
    jobs = []
    for i, ve in enumerate(va):
        n = -(-int(ve) // GMAX)
        for _ in range(n):
            jobs.append((ve / n, i))
    jobs.sort(key=lambda t: (-t[0], t[1]))
    S = -(-len(jobs) // N_CORES)
    N = np.zeros((len(va), S), dtype=int)
    for r in range(S):
        for _, i in jobs[N_CORES * r:N_CORES * (r + 1)]:
            N[i, r] += 1
    return N


def _optimize_topology(va, budget_s=25.0):
    """SA over bin topology with an exact LP for rank widths.
    Returns (N, L) with N[e,r] bins and L[r] float widths."""
    from scipy.optimize import linprog

    ne = len(va)
    S0 = _deal_jobs(va).shape[1]

    RANK_PENALTY = 12.0   # columns; discourages near-empty weight-load ranks

    def solve(N):
        S = N.shape[1]
        res = linprog(c=np.ones(S), A_ub=-N, b_ub=-va,
                      bounds=[(0, GMAX)] * S, method="highs")
        if not res.success:
            return None, np.inf
        used = int((N.sum(axis=0) > 0).sum())
        return res.x, float(res.x.sum()) + RANK_PENALTY * used

    rng = np.random.default_rng(0)
    t_start = time.time()
    best_global = None
    for idx, S in enumerate((S0, S0 + 1, S0 + 2)):
        slice_end = t_start + (idx + 1) * budget_s / 3.0
        N = np.zeros((ne, S), dtype=int)
        N0 = _deal_jobs(va)
        N[:, :N0.shape[1]] = N0
        L, cost = solve(N)
        if L is None:
            continue
        best = (cost, N.copy(), L.copy())
        T0, T1 = 30.0, 0.3
        iters = 40000
        for it in range(iters):
            if (it & 63) == 0 and time.time() > slice_end:
                break
            T = T0 * (T1 / T0) ** (it / iters)
            N2 = N.copy()
            kind = rng.random()
            if kind < 0.4:
                i = int(rng.integers(ne))
                rs = np.where(N2[i] > 0)[0]
                if len(rs) == 0:
                    continue
                r1 = int(rs[rng.integers(len(rs))])
                slack = np.where(N2.sum(axis=0) < N_CORES)[0]
                if len(slack) == 0:
                    continue
                r2 = int(slack[rng.integers(len(slack))])
                N2[i, r1] -= 1
                N2[i, r2] += 1
                if N2[i].sum() == 0:
                    continue
            elif kind < 0.6:
                i = int(rng.integers(ne))
                slack = np.where(N2.sum(axis=0) < N_CORES)[0]
                if len(slack) == 0:
                    continue
                r2 = int(slack[rng.integers(len(slack))])
                N2[i, r2] += 1
            elif kind < 0.75:
                i = int(rng.integers(ne))
                if N2[i].sum() <= 1:
                    continue
                rs = np.where(N2[i] > 0)[0]
                r1 = int(rs[rng.integers(len(rs))])
                N2[i, r1] -= 1
            else:
                # reassign one bin at a rank from expert i to expert j
                i = int(rng.integers(ne))
                rs = np.where(N2[i] > 0)[0]
                if len(rs) == 0 or N2[i].sum() <= 1:
                    continue
                r1 = int(rs[rng.integers(len(rs))])
                j = int(rng.integers(ne))
                if j == i:
                    continue
                N2[i, r1] -= 1
                N2[j, r1] += 1
            L2, c2 = solve(N2)
            if L2 is None:
                continue
            if c2 < cost or rng.random() < np.exp((cost - c2) / max(T, 1e-9)):
                N, cost, L = N2, c2, L2
                if cost < best[0]:
                    best = (cost, N.copy(), L.copy())
        if best_global is None or best[0] < best_global[0] - 1e-6:
            best_global = best
    _, N, L = best_global
    return N, L


def _plan(v):
    """Returns (slot_widths desc-sorted tuple, assign) where
    assign[core][slot] = (width, expert, col_start) or None."""
    act = [e for e in range(E) if int(v[e]) > 0]
    va = np.array([int(v[e]) for e in act], dtype=float)

    try:
        N, L = _optimize_topology(va)
    except Exception:
        N = _deal_jobs(va)
        from_max = []
        for r in range(N.shape[1]):
            need = [va[i] / N[i].sum() for i in range(len(va)) if N[i, r] > 0]
            from_max.append(max(need) if need else 0.0)
        L = np.array(from_max)

    S = N.shape[1]
    Lint = np.ceil(L - 1e-9).astype(int)

    # concrete piece widths: each expert consumes its bins largest-L first
    bins = [[] for _ in range(S)]          # rank -> list of (width, expert, col_start)
    for i, e in enumerate(act):
        rem = int(va[i])
        order = sorted(
            [r for r in range(S) for _ in range(N[i, r])],
            key=lambda r: -Lint[r])
        for r in order:
            w = min(Lint[r], rem)
            if w > 0:
                bins[r].append((w, e, int(va[i]) - rem))
                rem -= w
        assert rem == 0, "LP cover violated"

    # shrink each rank to its largest actual piece; drop empty ranks
    ranks = []
    for r in range(S):
        if bins[r]:
            ranks.append((max(w for w, _, _ in bins[r]),
                          sorted(bins[r], reverse=True)))
    ranks.sort(key=lambda t: -t[0])

    widths = tuple(t[0] for t in ranks)
    assign = [[None] * len(ranks) for _ in range(N_CORES)]
    for s, (_, blist) in enumerate(ranks):
        for c, piece in enumerate(blist):
            assign[c][s] = piece
    return widths, assign


def _build_program(slot_widths: tuple):
    """One SPMD program; slot s runs one piece of width slot_widths[s]."""
    nc = bacc.Bacc(None, target_bir_lowering=False)
    S = len(slot_widths)

    xt = nc.dram_tensor("xt", [S, D, GMAX], BF16, kind="ExternalInput")
    w1g = nc.dram_tensor("w1g", [S, D, H], BF16, kind="ExternalInput")
    w2g = nc.dram_tensor("w2g", [S, H, D], BF16, kind="ExternalInput")
    bg = nc.dram_tensor("bg", [S, 128, MT1 + MT2], F32, kind="ExternalInput")
    yt = nc.dram_tensor("yt", [S, D, GMAX], BF16, kind="ExternalOutput")

    Gelu = mybir.ActivationFunctionType.Gelu
    Ident = mybir.ActivationFunctionType.Identity

    with tile.TileContext(nc) as tc:
        with (
            tc.tile_pool(name="w1p", bufs=2) as w1p,
            tc.tile_pool(name="w2p", bufs=2) as w2p,
            tc.tile_pool(name="bp", bufs=1) as bp,
            tc.tile_pool(name="xp", bufs=3) as xp,
            tc.tile_pool(name="hp", bufs=2) as hp,
            tc.tile_pool(name="yp", bufs=2) as yp,
            tc.tile_pool(name="ps_h", bufs=4, space="PSUM") as ps_h,
            tc.tile_pool(name="ps_y", bufs=4, space="PSUM") as ps_y,
        ):
            # first slot: mid-size (~288 cols) so the critical fill moves
            # less x data while still covering the next slot's prefetch;
            # then interleave big/small (widths sorted descending by rank)
            # so DMA demand per compute window stays even; end with the
            # smallest slot so the pipeline drain tail is short
            if S > 2:
                first = min(range(S - 1),
                            key=lambda r: abs(slot_widths[r] - 288))
            else:
                first = 0
            rest = [r for r in range(S - 1) if r != first]
            emit_order = [first]
            lo, hi = 0, len(rest) - 1
            while lo <= hi:
                emit_order.append(rest[lo])
                if hi != lo:
                    emit_order.append(rest[hi])
                lo += 1
                hi -= 1
            if S > 1:
                emit_order.append(S - 1)
            # HAM pre-warm: ~3.4us of dummy matmuls (zeroed operand, result
            # discarded) run inside the DMA-fill dead time, so the first
            # real matmuls start at the un-throttled clock instead of
            # paying ~14 cold matmuls at 1.2GHz. The memset is the very
            # first gpsimd op so the dummies release right after the
            # engine preambles (~+6us), warming by the time data lands.
            warm_t = bp.tile([128, 256], BF16, tag="warm")
            nc.gpsimd.memset(warm_t, 0)
            ps_w = ps_h.tile([128, GMAX], F32, tag="psh")
            for _ in range(16):
                nc.tensor.matmul(
                    ps_w[:, :256], lhsT=warm_t[:, :128], rhs=warm_t,
                    start=True, stop=True,
                )

            # all biases in one small SWDGE transfer up front: keeps every
            # per-slot descriptor-gen off the HWDGE rings
            ball_t = bp.tile([128, S, MT1 + MT2], F32, tag="b")
            nc.gpsimd.dma_start(
                out=ball_t, in_=bg.rearrange("s p b -> p s b"))
            for si, s in enumerate(emit_order):
                W = slot_widths[s]
                b_t = ball_t[:, s]

                xt_s = xt[s].rearrange("(k p) c -> p k c", p=128)
                x_t = xp.tile([128, KT1, GMAX], BF16, tag="x")
                # x rides the ACT ring: the first matmul's two dependencies
                # (x and the first w1 piece) sit at the HEAD of the two
                # HWDGE FIFOs and stream in parallel; for the first slot,
                # split by contraction half so the very first matmuls wait
                # on even less data
                if si == 0:
                    nc.scalar.dma_start(out=x_t[:, :2, :W], in_=xt_s[:, :2, :W])
                    nc.scalar.dma_start(out=x_t[:, 2:, :W], in_=xt_s[:, 2:, :W])
                else:
                    nc.scalar.dma_start(out=x_t[:, :, :W], in_=xt_s[:, :, :W])

                w1_t = w1p.tile([128, KT1, H], BF16, tag="w1")
                w1_src = w1g[s].rearrange("(k p) h -> p k h", p=128)
                if si == 0:
                    # fine-grained first load so the first matmuls start
                    # after ~256KB; x streams in parallel on the other ring
                    nc.sync.dma_start(out=w1_t[:, :, :256], in_=w1_src[:, :, :256])
                    nc.sync.dma_start(out=w1_t[:, :, 256:1280], in_=w1_src[:, :, 256:1280])
                    nc.sync.dma_start(out=w1_t[:, :, 1280:], in_=w1_src[:, :, 1280:])
                else:
                    nc.sync.dma_start(out=w1_t[:, :, :H // 2], in_=w1_src[:, :, :H // 2])
                    nc.scalar.dma_start(out=w1_t[:, :, H // 2:], in_=w1_src[:, :, H // 2:])

                # w2 in halves on both HWDGE rings, behind w1 in each FIFO:
                # it is only needed once GEMM2 starts, and keeping all bulk
                # traffic on the two HWDGE FIFOs lets queue order prioritize
                # the critical path (SWDGE transfers would steal HBM bw
                # from the head-of-queue w1 pieces at kernel start)
                w2_t = w2p.tile([128, KT2, D], BF16, tag="w2")
                w2_src = w2g[s].rearrange("(k p) d -> p k d", p=128)
                nc.sync.dma_start(out=w2_t[:, :, :D // 2], in_=w2_src[:, :, :D // 2])
                nc.scalar.dma_start(out=w2_t[:, :, D // 2:], in_=w2_src[:, :, D // 2:])

                h_t = hp.tile([128, KT2, GMAX], BF16, tag="h")
                for m in range(MT1):
                    ps = ps_h.tile([128, GMAX], F32, tag="psh")
                    for k in range(KT1):
                        nc.tensor.matmul(
                            ps[:, :W],
                            lhsT=w1_t[:, k, m * 128:(m + 1) * 128],
                            rhs=x_t[:, k, :W],
                            start=(k == 0),
                            stop=(k == KT1 - 1),
                        )
                    nc.scalar.activation(
                        h_t[:, m, :W], ps[:, :W], Gelu, bias=b_t[:, m:m + 1]
                    )

                y_t = yp.tile([128, MT2, GMAX], BF16, tag="y")
                for dm in range(MT2):
                    ps2 = ps_y.tile([128, GMAX], F32, tag="psy")
                    for k in range(KT2):
                        nc.tensor.matmul(
                            ps2[:, :W],
                            lhsT=w2_t[:, k, dm * 128:(dm + 1) * 128],
                            rhs=h_t[:, k, :W],
                            start=(k == 0),
                            stop=(k == KT2 - 1),
                        )
                    nc.scalar.activation(
                        y_t[:, dm, :W], ps2[:, :W], Ident,
                        bias=b_t[:, MT1 + dm:MT1 + dm + 1]
                    )
                yt_s = yt[s].rearrange("(m p) c -> p m c", p=128)
                # the last slot's output goes out on the (by then idle)
                # HWDGE sync ring: shorter completion latency than SWDGE,
                # so the end-of-program drain is shorter
                if si == len(emit_order) - 1:
                    nc.sync.dma_start(out=yt_s[:, :, :W], in_=y_t[:, :, :W])
                else:
                    nc.gpsimd.dma_start(out=yt_s[:, :, :W], in_=y_t[:, :, :W])

    nc.compile()
    return nc


def kernel(packed_inputs, valid_load, w1, b1, w2, b2, _trace=False, **_):
    global LAST_RESULT
    x = np.ascontiguousarray(np.asarray(packed_inputs, np.float32))
    w1 = np.asarray(w1, np.float32)
    b1 = np.asarray(b1, np.float32)
    w2 = np.asarray(w2, np.float32)
    b2 = np.asarray(b2, np.float32)
    v = np.asarray(valid_load).astype(np.int64)

    out = np.zeros((E, CAP, D), np.float32)
    if int(v.max()) <= 0:
        return out

    slot_widths, assign = _plan(v)
    S = len(slot_widths)

    key = slot_widths
    if key not in _PROGRAM_CACHE:
        _PROGRAM_CACHE[key] = _build_program(slot_widths)
    nc = _PROGRAM_CACHE[key]

    w1b = w1.astype(NPBF16)
    w2b = w2.astype(NPBF16)
    br = np.concatenate([
        b1.reshape(E, MT1, 128).transpose(0, 2, 1),
        b2.reshape(E, MT2, 128).transpose(0, 2, 1),
    ], axis=2)

    in_maps = []
    for c in range(N_CORES):
        xt = np.zeros((S, D, GMAX), NPBF16)
        w1g = np.empty((S, D, H), NPBF16)
        w2g = np.empty((S, H, D), NPBF16)
        bg = np.zeros((S, 128, MT1 + MT2), np.float32)
        for s in range(S):
            job = assign[c][s]
            if job is None:
                w1g[s] = w1b[0]
                w2g[s] = w2b[0]
                continue
            w, e, cs = job
            xt[s, :, :w] = x[e, cs:cs + w, :].T.astype(NPBF16)
            w1g[s] = w1b[e]
            w2g[s] = w2b[e]
            bg[s] = br[e]
        in_maps.append({"xt": xt, "w1g": w1g, "w2g": w2g, "bg": bg})

    res = run_bass_kernel_spmd(nc, in_maps, list(range(N_CORES)), trace=_trace)
    LAST_RESULT = res

    for c in range(N_CORES):
        ytc = res.results[c]["yt"]
        for s in range(S):
            job = assign[c][s]
            if job is None:
                continue
            w, e, cs = job
            out[e, cs:cs + w, :] = ytc[s, :, :w].T.astype(np.float32)
    return out
